# revision 30
# baseline (speedup 1.0000x reference)
"""2-layer GCN (PyG GCNConv semantics) on 8 TRN2 NeuronCores.

Strategy (per the sharding hint):
- Nodes are sharded contiguously across the 8 cores (12500 rows each).
- Within each core, local nodes are sorted by degree (descending) so the
  per-128-node-tile padded neighbor-slot grids are tight. The permutation is
  undone on the host after the run.
- Layer math is refactored as: h~ = D^-1/2 (x @ W1); table = AllGather(h~);
  agg[d] = D^-1/2[d] * sum_{e: dst=d} table[src[e]]  (self-loops are edges).
  Layer 2 uses agg2 = D^-1/2 * segsum(AllGather(D^-1/2 relu(agg1 + b1))),
  then out = log_softmax(agg2 @ W2 + b2)   [valid since A(hW) = (Ah)W].
- The irregular gather reads an AllGather'ed bf16 node-feature table in HBM
  via GPSIMD indirect DMA, one [128,1]-offset instruction per slot column
  (the HW vector-indirect DMA honors exactly one offset per output
  partition-row); the per-destination segment-sum is a strided DVE
  tensor_reduce over a padded slot grid (pad slots point at a zero row
  appended to the table). Same-SWDGE-queue flush DMAs provide reliable
  gather-completion points, and slot-buffer reuse is gated on them.
- log_softmax's ln() is computed by Newton iteration on ScalarE Exp (no Ln
  in any ACT table set); Relu runs on the exact DVE ALU path.
"""
import sys

sys.path.insert(0, "/opt/trn_rl_repo")

import numpy as np

import concourse.bass as bass
import concourse.bacc as bacc
import concourse.tile as tile
import concourse.mybir as mybir
from concourse import bass_utils
from concourse.masks import make_identity
from concourse.tile import add_dep_helper

N = 100000
F = 512
H = 64
CLS = 10
NC = 8
NPC = N // NC          # 12500 nodes per core
P = 128
NT = (NPC + P - 1) // P  # 98 tiles per core
NPAD = NT * P            # 12544
RG = [list(range(NC))]

BF16 = mybir.dt.bfloat16
F32 = mybir.dt.float32
I32 = mybir.dt.int32
NP_BF16 = mybir.dt.np(BF16)

IDX_BUFS = 2
MSG_BUFS = 5
RING_DESCS = 6200

_cache = {}


def _build(Wt):
    """Build + compile the SPMD graph for a per-tile slot-width schedule Wt."""
    Wt = list(Wt)
    S = int(P * sum(Wt))
    nc = bacc.Bacc("TRN2", target_bir_lowering=False, debug=False, num_devices=NC,
                   num_swdge_queues=2)

    x_in = nc.dram_tensor("x_in", [NPAD, F], BF16, kind="ExternalInput")
    w1_in = nc.dram_tensor("w1_in", [F, H], BF16, kind="ExternalInput")
    b1_in = nc.dram_tensor("b1_in", [P, H], F32, kind="ExternalInput")
    w2_in = nc.dram_tensor("w2_in", [H, CLS], BF16, kind="ExternalInput")
    b2_in = nc.dram_tensor("b2_in", [P, CLS], F32, kind="ExternalInput")
    dinv_in = nc.dram_tensor("dinv_in", [P, NT], F32, kind="ExternalInput")
    SW = S // P
    idx_in = nc.dram_tensor("idx_in", [P, SW], I32, kind="ExternalInput")
    out = nc.dram_tensor("out", [NPAD, CLS], F32, kind="ExternalOutput")

    bounce1 = nc.dram_tensor("bounce1", [NPC, H], BF16, kind="Internal")
    bounce2 = nc.dram_tensor("bounce2", [NPC, H], BF16, kind="Internal")
    table1 = nc.dram_tensor("table1", [N + 1, H], BF16, kind="Internal",
                            addr_space="Shared")
    table2 = nc.dram_tensor("table2", [N + 1, H], BF16, kind="Internal",
                            addr_space="Shared")

    AF = mybir.ActivationFunctionType
    ALU = mybir.AluOpType

    with tile.TileContext(nc) as tc:
        with (
            tc.tile_pool(name="const", bufs=1) as constp,
            tc.tile_pool(name="xt", bufs=8) as xtp,
            tc.tile_pool(name="hpsum", bufs=3, space="PSUM") as hpsum,
            tc.tile_pool(name="hsb", bufs=4) as hsb,
            tc.tile_pool(name="idxp", bufs=IDX_BUFS) as idxp,
            tc.tile_pool(name="msgp", bufs=MSG_BUFS) as msgp,
            tc.tile_pool(name="aggp", bufs=4) as aggp,
            tc.tile_pool(name="smp", bufs=4) as smp,
            tc.tile_pool(name="psum2", bufs=2, space="PSUM") as psum2,
        ):
            # --- constants ---
            w1s = constp.tile([P, 4 * H], BF16)  # chunk k at [:, k*H:(k+1)*H]
            for k in range(4):
                nc.sync.dma_start(w1s[:, k * H:(k + 1) * H],
                                  w1_in.ap()[k * P:(k + 1) * P, :])
            b1s = constp.tile([P, H], F32)
            nc.sync.dma_start(b1s[:], b1_in.ap())
            w2s = constp.tile([H, CLS], BF16)
            nc.sync.dma_start(w2s[:], w2_in.ap())
            b2s = constp.tile([P, CLS], F32)
            nc.sync.dma_start(b2s[:], b2_in.ap())
            dinvs = constp.tile([P, NT], F32)
            nc.sync.dma_start(dinvs[:], dinv_in.ap())
            ident = constp.tile([P, P], BF16)
            make_identity(nc, ident[:])
            zrow = constp.tile([1, H], BF16)
            nc.gpsimd.memset(zrow[:], 0.0)
            flsrc = constp.tile([P, 16], F32)
            nc.gpsimd.memset(flsrc[:], 0.0)
            idxall = constp.tile([P, SW], I32)
            nc.sync.dma_start(idxall[:], idx_in.ap())

            def flushed_gather(msg, table_ap, idxt, flpool, W, q=0):
                """Indirect row-gather, one [P,1] DMA per slot column.

                The HW vector-indirect DMA only honors one offset per output
                partition-row, so gather W slot columns with W separate
                instructions. A trailing same-SWDGE-queue flush DMA provides
                a reliable completion point (the gathers' own semaphores can
                fire before all descriptors land). Returns (first, flush).
                """
                g = None
                qn = "qPoolDynamic1" if q else "qPoolDynamic"
                for j in range(1, W):
                    gi = nc.gpsimd.indirect_dma_start(
                        out=msg[:, j * H:(j + 1) * H], out_offset=None,
                        in_=table_ap,
                        in_offset=bass.IndirectOffsetOnAxis(
                            ap=idxt[:, j:j + 1], axis=0))
                    gi.ins.queue = qn
                    if g is None:
                        g = gi
                fldst = flpool.tile([P, 16], F32, tag="fldst")
                fl = nc.gpsimd.dma_start(fldst[:], flsrc[:])
                fl.ins.queue = qn
                if g is not None:
                    add_dep_helper(fl.ins, gi.ins,
                                   reason="flush follows gathers in ring")
                else:
                    g = fl
                return g, fl
            nc.sync.dma_start(table1.ap()[N:N + 1, :], zrow[:])
            nc.sync.dma_start(table2.ap()[N:N + 1, :], zrow[:])

            # --- phase A: h1~ = dinv * (x @ W1), write local shard to bounce1
            for t in range(NT):
                rows = min(P, NPC - t * P)
                ps = hpsum.tile([P, H], F32, tag="hps")
                for k in range(4):
                    xt = xtp.tile([P, P], BF16, tag="xt")
                    nc.sync.dma_start_transpose(
                        xt[:], x_in.ap()[t * P:(t + 1) * P, k * P:(k + 1) * P])
                    nc.tensor.matmul(ps[:], lhsT=xt[:],
                                     rhs=w1s[:, k * H:(k + 1) * H],
                                     start=(k == 0), stop=(k == 3))
                h1 = hsb.tile([P, H], BF16, tag="h1")
                nc.vector.tensor_scalar(out=h1[:], in0=ps[:],
                                        scalar1=dinvs[:, t:t + 1], scalar2=None,
                                        op0=ALU.mult)
                nc.sync.dma_start(bounce1.ap()[t * P:t * P + rows, :], h1[:rows, :])

            nc.gpsimd.collective_compute(
                "AllGather", ALU.bypass, replica_groups=RG,
                ins=[bounce1.ap()[:].opt()], outs=[table1.ap()[0:N, :].opt()])

            # --- phase C: layer-1 aggregation ---
            off = 0
            fls = []
            flsq = {0: [], 1: []}
            for t in range(NT):
                rows = min(P, NPC - t * P)
                W = Wt[t]
                msg = msgp.tile([P, W * H], BF16, tag="msg")
                sd = nc.sync.dma_start(msg[:rows, 0:H],
                                       bounce1.ap()[t * P:t * P + rows, :])
                q = t % 2
                g, fl = flushed_gather(msg, table1.ap()[:],
                                       idxall[:, off // P:off // P + W], smp, W,
                                       q=q)
                if t >= MSG_BUFS:
                    add_dep_helper(g.ins, fls[t - MSG_BUFS].ins,
                                   reason="msg slot reuse waits gather flush")
                    add_dep_helper(sd.ins, fls[t - MSG_BUFS].ins,
                                   reason="self col slot reuse waits flush")
                myq = flsq[q]
                acc = W * P
                k = len(myq) - 1
                while k >= 0 and acc + myq[k][1] <= RING_DESCS:
                    acc += myq[k][1]
                    k -= 1
                if k >= 0:
                    add_dep_helper(g.ins, myq[k][0].ins, reason="ring window")
                fls.append(fl)
                flsq[q].append((fl, W * P))
                agg = aggp.tile([P, H], F32, tag="agg")
                red = nc.vector.tensor_reduce(
                    out=agg[:], in_=msg[:].rearrange("p (j f) -> p f j", f=H),
                    axis=mybir.AxisListType.X, op=ALU.add)
                add_dep_helper(red.ins, fl.ins, reason="reduce waits gather flush")
                y = aggp.tile([P, H], F32, tag="y")
                nc.vector.tensor_scalar(out=y[:], in0=agg[:],
                                        scalar1=dinvs[:, t:t + 1], scalar2=None,
                                        op0=ALU.mult)
                y2 = aggp.tile([P, H], F32, tag="y2")
                nc.vector.tensor_tensor(out=y2[:], in0=y[:], in1=b1s[:], op=ALU.add)
                h2 = hsb.tile([P, H], BF16, tag="h2")
                nc.vector.tensor_scalar(out=h2[:], in0=y2[:], scalar1=0.0,
                                        scalar2=dinvs[:, t:t + 1],
                                        op0=ALU.max, op1=ALU.mult)
                nc.sync.dma_start(bounce2.ap()[t * P:t * P + rows, :], h2[:rows, :])
                off += P * W

            nc.gpsimd.collective_compute(
                "AllGather", ALU.bypass, replica_groups=RG,
                ins=[bounce2.ap()[:].opt()], outs=[table2.ap()[0:N, :].opt()])

            # --- phase E: layer-2 aggregation + GEMM2 + log_softmax ---
            off = 0
            for t in range(NT):
                W = Wt[t]
                tt = NT + t
                rows = min(P, NPC - t * P)
                msg = msgp.tile([P, W * H], BF16, tag="msg")
                sd = nc.sync.dma_start(msg[:rows, 0:H],
                                       bounce2.ap()[t * P:t * P + rows, :])
                q = tt % 2
                g, fl = flushed_gather(msg, table2.ap()[:],
                                       idxall[:, off // P:off // P + W], smp, W,
                                       q=q)
                if tt >= MSG_BUFS:
                    add_dep_helper(g.ins, fls[tt - MSG_BUFS].ins,
                                   reason="msg slot reuse waits gather flush")
                    add_dep_helper(sd.ins, fls[tt - MSG_BUFS].ins,
                                   reason="self col slot reuse waits flush")
                myq = flsq[q]
                acc = W * P
                k = len(myq) - 1
                while k >= 0 and acc + myq[k][1] <= RING_DESCS:
                    acc += myq[k][1]
                    k -= 1
                if k >= 0:
                    add_dep_helper(g.ins, myq[k][0].ins, reason="ring window")
                fls.append(fl)
                flsq[q].append((fl, W * P))
                agg = aggp.tile([P, H], F32, tag="agg")
                red = nc.vector.tensor_reduce(
                    out=agg[:], in_=msg[:].rearrange("p (j f) -> p f j", f=H),
                    axis=mybir.AxisListType.X, op=ALU.add)
                add_dep_helper(red.ins, fl.ins, reason="reduce waits gather flush")
                aggb = smp.tile([P, H], BF16, tag="aggb")
                nc.vector.tensor_scalar(out=aggb[:], in0=agg[:],
                                        scalar1=dinvs[:, t:t + 1], scalar2=None,
                                        op0=ALU.mult)
                pt = psum2.tile([H, P], BF16, tag="pt")
                nc.tensor.transpose(out=pt[:], in_=aggb[:], identity=ident[:])
                aggT = smp.tile([H, P], BF16, tag="aggT")
                nc.vector.tensor_copy(out=aggT[:], in_=pt[:])
                po = psum2.tile([P, CLS], F32, tag="po")
                nc.tensor.matmul(po[:], lhsT=aggT[:], rhs=w2s[:], start=True, stop=True)
                yo = smp.tile([P, CLS], F32, tag="yo")
                nc.vector.tensor_tensor(out=yo[:], in0=po[:], in1=b2s[:], op=ALU.add)
                mx = smp.tile([P, 1], F32, tag="mx")
                nc.vector.tensor_reduce(out=mx[:], in_=yo[:],
                                        axis=mybir.AxisListType.X, op=ALU.max)
                sh = smp.tile([P, CLS], F32, tag="sh")
                nc.vector.tensor_scalar(out=sh[:], in0=yo[:], scalar1=mx[:, 0:1],
                                        scalar2=None, op0=ALU.subtract)
                ex = smp.tile([P, CLS], F32, tag="ex")
                nc.scalar.activation(out=ex[:], in_=sh[:], func=AF.Exp)
                sm = smp.tile([P, 1], F32, tag="sm")
                nc.vector.tensor_reduce(out=sm[:], in_=ex[:],
                                        axis=mybir.AxisListType.X, op=ALU.add)
                # ls = ln(sm), sm in [1, CLS]; no Ln in any ACT table set, so
                # Newton on f(y) = e^y - sm:  y <- y - 1 + sm * e^-y
                ls = smp.tile([P, 1], F32, tag="ls")
                nc.vector.tensor_scalar(out=ls[:], in0=sm[:], scalar1=0.2559,
                                        scalar2=-0.2559, op0=ALU.mult, op1=ALU.add)
                for _ in range(4):
                    en = smp.tile([P, 1], F32, tag="en")
                    nc.scalar.activation(out=en[:], in_=ls[:], func=AF.Exp,
                                         scale=-1.0)
                    pr = smp.tile([P, 1], F32, tag="pr")
                    nc.vector.tensor_tensor(out=pr[:], in0=en[:], in1=sm[:],
                                            op=ALU.mult)
                    ls2 = smp.tile([P, 1], F32, tag="ls")
                    nc.vector.tensor_tensor(out=ls2[:], in0=ls[:], in1=pr[:],
                                            op=ALU.add)
                    ls = ls2
                    nc.vector.tensor_scalar(out=ls[:], in0=ls[:], scalar1=1.0,
                                            scalar2=None, op0=ALU.subtract)
                res = smp.tile([P, CLS], F32, tag="res")
                nc.vector.tensor_scalar(out=res[:], in0=sh[:], scalar1=ls[:, 0:1],
                                        scalar2=None, op0=ALU.subtract)
                nc.sync.dma_start(out.ap()[t * P:(t + 1) * P, :], res[:])
                off += P * W

    nc.compile()
    return nc


def _prep(x, edge_index, W1, b1, W2, b2):
    """Host-side graph preprocessing; returns (Wt schedule, per-core in_maps, order)."""
    x = np.asarray(x, dtype=np.float32)
    ei = np.asarray(edge_index, dtype=np.int64)
    W1 = np.asarray(W1, dtype=np.float32)
    b1 = np.asarray(b1, dtype=np.float32)
    W2 = np.asarray(W2, dtype=np.float32)
    b2 = np.asarray(b2, dtype=np.float32)

    nodes = np.arange(N, dtype=np.int64)
    src_f = np.concatenate([nodes, ei[0]])
    dst_f = np.concatenate([nodes, ei[1]])
    deg = np.bincount(dst_f, minlength=N)  # >= 1 (self-loops)
    dinv = (1.0 / np.sqrt(deg)).astype(np.float32)

    # per-core degree-descending permutation
    order = np.argsort(-deg.reshape(NC, NPC), axis=1, kind="stable")  # [NC, NPC]
    perm_global = (np.arange(NC, dtype=np.int64)[:, None] * NPC + order)  # [NC, NPC]
    pos_of = np.empty(N, np.int64)
    pos_of[perm_global.ravel()] = np.arange(N, dtype=np.int64)

    dpos = pos_of[dst_f]
    spos = pos_of[src_f]
    sidx = np.argsort(dpos, kind="stable")
    dpos_s = dpos[sidx]
    spos_s = spos[sidx]
    cnt = np.bincount(dpos_s, minlength=N)

    cnt_t = cnt.reshape(NC, NPC)
    cnt_pad = np.zeros((NC, NPAD), np.int64)
    cnt_pad[:, :NPC] = cnt_t
    Wt = np.maximum(cnt_pad.reshape(NC, NT, P).max(axis=2).max(axis=0), 1)  # [NT]
    off_t = np.zeros(NT + 1, np.int64)
    np.cumsum(P * Wt, out=off_t[1:])
    S = int(off_t[-1])

    starts = np.zeros(N + 1, np.int64)
    np.cumsum(cnt, out=starts[1:])
    j = np.arange(len(dpos_s), dtype=np.int64) - starts[dpos_s]
    c_arr = dpos_s // NPC
    il = dpos_s % NPC
    t_arr = il // P
    p_arr = il % P
    col_of = off_t // P  # column offset of each tile in the [P, SW] buffer
    col = col_of[t_arr] + j
    SW = S // P
    idx_all = np.full((NC, P, SW), N, dtype=np.int32)
    idx_all[c_arr, p_arr, col] = spos_s.astype(np.int32)

    dinv_perm = dinv[perm_global]  # [NC, NPC]
    dinv_pad = np.ones((NC, NPAD), np.float32)
    dinv_pad[:, :NPC] = dinv_perm
    dinvT = np.ascontiguousarray(
        dinv_pad.reshape(NC, NT, P).transpose(0, 2, 1))  # [NC, P, NT]

    W1_bf = np.ascontiguousarray(W1.astype(NP_BF16))
    W2_bf = np.ascontiguousarray(W2.astype(NP_BF16))
    b1_bc = np.ascontiguousarray(np.broadcast_to(b1[None, :], (P, H)).astype(np.float32))
    b2_bc = np.ascontiguousarray(np.broadcast_to(b2[None, :], (P, CLS)).astype(np.float32))

    in_maps = []
    for c in range(NC):
        xp = np.zeros((NPAD, F), dtype=NP_BF16)
        xp[:NPC] = x[perm_global[c]].astype(NP_BF16)
        in_maps.append({
            "x_in": xp,
            "w1_in": W1_bf,
            "b1_in": b1_bc,
            "w2_in": W2_bf,
            "b2_in": b2_bc,
            "dinv_in": np.ascontiguousarray(dinvT[c]),
            "idx_in": np.ascontiguousarray(idx_all[c]),
        })
    return tuple(int(w) for w in Wt), in_maps, order


def _get_nc(Wt):
    if Wt not in _cache:
        _cache[Wt] = _build(Wt)
    return _cache[Wt]


def run(x, edge_index, W1, b1, W2, b2, trace=False):
    Wt, in_maps, order = _prep(x, edge_index, W1, b1, W2, b2)
    nc = _get_nc(Wt)
    res = bass_utils.run_bass_kernel_spmd(
        nc, in_maps, core_ids=list(range(NC)), trace=trace)
    out_full = np.empty((N, CLS), np.float32)
    for c in range(NC):
        oc = res.results[c]["out"][:NPC]
        out_full[c * NPC + order[c]] = oc
    return out_full, res


def kernel(x, edge_index, W1, b1, W2, b2):
    out_full, _ = run(x, edge_index, W1, b1, W2, b2)
    return out_full


# revision 31
# speedup vs baseline: 1.1741x; 1.1741x over previous
"""2-layer GCN (PyG GCNConv semantics) on 8 TRN2 NeuronCores.

Strategy (per the sharding hint):
- Nodes are sharded contiguously across the 8 cores (12500 rows each).
- Within each core, local nodes are sorted by degree (descending) so the
  per-128-node-tile padded neighbor-slot grids are tight. The permutation is
  undone on the host after the run.
- Layer math is refactored as: h~ = D^-1/2 (x @ W1); table = AllGather(h~);
  agg[d] = D^-1/2[d] * sum_{e: dst=d} table[src[e]]  (self-loops are edges).
  Layer 2 uses agg2 = D^-1/2 * segsum(AllGather(D^-1/2 relu(agg1 + b1))),
  then out = log_softmax(agg2 @ W2 + b2)   [valid since A(hW) = (Ah)W].
- The irregular gather reads an AllGather'ed bf16 node-feature table in HBM
  via GPSIMD indirect DMA, one [128,1]-offset instruction per slot column
  (the HW vector-indirect DMA honors exactly one offset per output
  partition-row); the per-destination segment-sum is a strided DVE
  tensor_reduce over a padded slot grid (pad slots point at a zero row
  appended to the table). Same-SWDGE-queue flush DMAs provide reliable
  gather-completion points, and slot-buffer reuse is gated on them.
- log_softmax's ln() is computed by Newton iteration on ScalarE Exp (no Ln
  in any ACT table set); Relu runs on the exact DVE ALU path.
"""
import sys

sys.path.insert(0, "/opt/trn_rl_repo")

import numpy as np

import concourse.bass as bass
import concourse.bacc as bacc
import concourse.tile as tile
import concourse.mybir as mybir
from concourse import bass_utils
from concourse.masks import make_identity
from concourse.tile import add_dep_helper

N = 100000
F = 512
H = 64
CLS = 10
NC = 8
NPC = N // NC          # 12500 nodes per core
P = 128
NT = (NPC + P - 1) // P  # 98 tiles per core
NPAD = NT * P            # 12544
RG = [list(range(NC))]

BF16 = mybir.dt.bfloat16
F32 = mybir.dt.float32
I32 = mybir.dt.int32
NP_BF16 = mybir.dt.np(BF16)

IDX_BUFS = 2
MSG_BUFS = 5
RING_DESCS = 14000

_cache = {}


def _build(Wt):
    """Build + compile the SPMD graph for a per-tile slot-width schedule Wt."""
    Wt = list(Wt)
    S = int(P * sum(Wt))
    nc = bacc.Bacc("TRN2", target_bir_lowering=False, debug=False, num_devices=NC)

    x_in = nc.dram_tensor("x_in", [NPAD, F], BF16, kind="ExternalInput")
    w1_in = nc.dram_tensor("w1_in", [F, H], BF16, kind="ExternalInput")
    b1_in = nc.dram_tensor("b1_in", [P, H], F32, kind="ExternalInput")
    w2_in = nc.dram_tensor("w2_in", [H, CLS], BF16, kind="ExternalInput")
    b2_in = nc.dram_tensor("b2_in", [P, CLS], F32, kind="ExternalInput")
    dinv_in = nc.dram_tensor("dinv_in", [P, NT], F32, kind="ExternalInput")
    SW = S // P
    idx_in = nc.dram_tensor("idx_in", [P, SW], I32, kind="ExternalInput")
    out = nc.dram_tensor("out", [NPAD, CLS], F32, kind="ExternalOutput")

    bounce1 = nc.dram_tensor("bounce1", [NPC, H], BF16, kind="Internal")
    bounce2 = nc.dram_tensor("bounce2", [NPC, H], BF16, kind="Internal")
    table1 = nc.dram_tensor("table1", [N + 1, H], BF16, kind="Internal",
                            addr_space="Shared")
    table2 = nc.dram_tensor("table2", [N + 1, H], BF16, kind="Internal",
                            addr_space="Shared")

    AF = mybir.ActivationFunctionType
    ALU = mybir.AluOpType

    with tile.TileContext(nc) as tc:
        with (
            tc.tile_pool(name="const", bufs=1) as constp,
            tc.tile_pool(name="xt", bufs=8) as xtp,
            tc.tile_pool(name="hpsum", bufs=3, space="PSUM") as hpsum,
            tc.tile_pool(name="hsb", bufs=4) as hsb,
            tc.tile_pool(name="idxp", bufs=IDX_BUFS) as idxp,
            tc.tile_pool(name="msgp", bufs=MSG_BUFS) as msgp,
            tc.tile_pool(name="aggp", bufs=4) as aggp,
            tc.tile_pool(name="smp", bufs=4) as smp,
            tc.tile_pool(name="psum2", bufs=2, space="PSUM") as psum2,
        ):
            # --- constants ---
            w1s = constp.tile([P, 4 * H], BF16)  # chunk k at [:, k*H:(k+1)*H]
            for k in range(4):
                nc.sync.dma_start(w1s[:, k * H:(k + 1) * H],
                                  w1_in.ap()[k * P:(k + 1) * P, :])
            b1s = constp.tile([P, H], F32)
            nc.sync.dma_start(b1s[:], b1_in.ap())
            w2s = constp.tile([H, CLS], BF16)
            nc.sync.dma_start(w2s[:], w2_in.ap())
            b2s = constp.tile([P, CLS], F32)
            nc.sync.dma_start(b2s[:], b2_in.ap())
            dinvs = constp.tile([P, NT], F32)
            nc.sync.dma_start(dinvs[:], dinv_in.ap())
            ident = constp.tile([P, P], BF16)
            make_identity(nc, ident[:])
            zrow = constp.tile([1, H], BF16)
            nc.gpsimd.memset(zrow[:], 0.0)
            flsrc = constp.tile([P, 16], F32)
            nc.gpsimd.memset(flsrc[:], 0.0)
            idxall = constp.tile([P, SW], I32)
            nc.sync.dma_start(idxall[:], idx_in.ap())

            def flushed_gather(msg, table_ap, idxt, flpool, W):
                """Indirect row-gather, one [P,1] DMA per slot column.

                The HW vector-indirect DMA only honors one offset per output
                partition-row, so gather W slot columns with W separate
                instructions. A trailing same-SWDGE-queue flush DMA provides
                a reliable completion point (the gathers' own semaphores can
                fire before all descriptors land). Returns (first, flush).
                """
                g = None
                for j in range(1, W):
                    gi = nc.gpsimd.indirect_dma_start(
                        out=msg[:, j * H:(j + 1) * H], out_offset=None,
                        in_=table_ap,
                        in_offset=bass.IndirectOffsetOnAxis(
                            ap=idxt[:, j:j + 1], axis=0))
                    if g is None:
                        g = gi
                fldst = flpool.tile([P, 16], F32, tag="fldst")
                fl = nc.gpsimd.dma_start(fldst[:], flsrc[:])
                if g is not None:
                    add_dep_helper(fl.ins, gi.ins,
                                   reason="flush follows gathers in ring")
                else:
                    g = fl
                return g, fl
            nc.sync.dma_start(table1.ap()[N:N + 1, :], zrow[:])
            nc.sync.dma_start(table2.ap()[N:N + 1, :], zrow[:])

            # --- phase A: h1~ = dinv * (x @ W1), write local shard to bounce1
            for t in range(NT):
                rows = min(P, NPC - t * P)
                ps = hpsum.tile([P, H], F32, tag="hps")
                for k in range(4):
                    xt = xtp.tile([P, P], BF16, tag="xt")
                    nc.sync.dma_start_transpose(
                        xt[:], x_in.ap()[t * P:(t + 1) * P, k * P:(k + 1) * P])
                    nc.tensor.matmul(ps[:], lhsT=xt[:],
                                     rhs=w1s[:, k * H:(k + 1) * H],
                                     start=(k == 0), stop=(k == 3))
                h1 = hsb.tile([P, H], BF16, tag="h1")
                nc.vector.tensor_scalar(out=h1[:], in0=ps[:],
                                        scalar1=dinvs[:, t:t + 1], scalar2=None,
                                        op0=ALU.mult)
                nc.sync.dma_start(bounce1.ap()[t * P:t * P + rows, :], h1[:rows, :])

            nc.gpsimd.collective_compute(
                "AllGather", ALU.bypass, replica_groups=RG,
                ins=[bounce1.ap()[:].opt()], outs=[table1.ap()[0:N, :].opt()])

            # --- phase C: layer-1 aggregation ---
            off = 0
            fls = []
            for t in range(NT):
                rows = min(P, NPC - t * P)
                W = Wt[t]
                msg = msgp.tile([P, W * H], BF16, tag="msg")
                sd = nc.sync.dma_start(msg[:rows, 0:H],
                                       bounce1.ap()[t * P:t * P + rows, :])
                g, fl = flushed_gather(msg, table1.ap()[:],
                                       idxall[:, off // P:off // P + W], smp, W)
                if t >= MSG_BUFS:
                    add_dep_helper(g.ins, fls[t - MSG_BUFS].ins,
                                   reason="msg slot reuse waits gather flush")
                    add_dep_helper(sd.ins, fls[t - MSG_BUFS].ins,
                                   reason="self col slot reuse waits flush")
                # SWDGE ring bound: keep in-flight descriptors under the
                # carveout by waiting on the newest flush that frees room
                acc = W * P
                k = t - 1
                while k >= 0 and acc + Wt[k] * P <= RING_DESCS:
                    acc += Wt[k] * P
                    k -= 1
                if k >= 0:
                    add_dep_helper(g.ins, fls[k].ins, reason="ring window")
                fls.append(fl)
                agg = aggp.tile([P, H], F32, tag="agg")
                red = nc.vector.tensor_reduce(
                    out=agg[:], in_=msg[:].rearrange("p (j f) -> p f j", f=H),
                    axis=mybir.AxisListType.X, op=ALU.add)
                add_dep_helper(red.ins, fl.ins, reason="reduce waits gather flush")
                y = aggp.tile([P, H], F32, tag="y")
                nc.vector.tensor_scalar(out=y[:], in0=agg[:],
                                        scalar1=dinvs[:, t:t + 1], scalar2=None,
                                        op0=ALU.mult)
                y2 = aggp.tile([P, H], F32, tag="y2")
                nc.vector.tensor_tensor(out=y2[:], in0=y[:], in1=b1s[:], op=ALU.add)
                h2 = hsb.tile([P, H], BF16, tag="h2")
                nc.vector.tensor_scalar(out=h2[:], in0=y2[:], scalar1=0.0,
                                        scalar2=dinvs[:, t:t + 1],
                                        op0=ALU.max, op1=ALU.mult)
                nc.sync.dma_start(bounce2.ap()[t * P:t * P + rows, :], h2[:rows, :])
                off += P * W

            nc.gpsimd.collective_compute(
                "AllGather", ALU.bypass, replica_groups=RG,
                ins=[bounce2.ap()[:].opt()], outs=[table2.ap()[0:N, :].opt()])

            # --- phase E: layer-2 aggregation + GEMM2 + log_softmax ---
            off = 0
            for t in range(NT):
                W = Wt[t]
                tt = NT + t
                rows = min(P, NPC - t * P)
                msg = msgp.tile([P, W * H], BF16, tag="msg")
                sd = nc.sync.dma_start(msg[:rows, 0:H],
                                       bounce2.ap()[t * P:t * P + rows, :])
                g, fl = flushed_gather(msg, table2.ap()[:],
                                       idxall[:, off // P:off // P + W], smp, W)
                if tt >= MSG_BUFS:
                    add_dep_helper(g.ins, fls[tt - MSG_BUFS].ins,
                                   reason="msg slot reuse waits gather flush")
                    add_dep_helper(sd.ins, fls[tt - MSG_BUFS].ins,
                                   reason="self col slot reuse waits flush")
                acc = W * P
                k = tt - 1
                while k >= 0 and acc + Wt[k % NT] * P <= RING_DESCS:
                    acc += Wt[k % NT] * P
                    k -= 1
                if k >= 0:
                    add_dep_helper(g.ins, fls[k].ins, reason="ring window")
                fls.append(fl)
                agg = aggp.tile([P, H], F32, tag="agg")
                red = nc.vector.tensor_reduce(
                    out=agg[:], in_=msg[:].rearrange("p (j f) -> p f j", f=H),
                    axis=mybir.AxisListType.X, op=ALU.add)
                add_dep_helper(red.ins, fl.ins, reason="reduce waits gather flush")
                aggb = smp.tile([P, H], BF16, tag="aggb")
                nc.vector.tensor_scalar(out=aggb[:], in0=agg[:],
                                        scalar1=dinvs[:, t:t + 1], scalar2=None,
                                        op0=ALU.mult)
                pt = psum2.tile([H, P], BF16, tag="pt")
                nc.tensor.transpose(out=pt[:], in_=aggb[:], identity=ident[:])
                aggT = smp.tile([H, P], BF16, tag="aggT")
                nc.vector.tensor_copy(out=aggT[:], in_=pt[:])
                po = psum2.tile([P, CLS], F32, tag="po")
                nc.tensor.matmul(po[:], lhsT=aggT[:], rhs=w2s[:], start=True, stop=True)
                yo = smp.tile([P, CLS], F32, tag="yo")
                nc.vector.tensor_tensor(out=yo[:], in0=po[:], in1=b2s[:], op=ALU.add)
                mx = smp.tile([P, 1], F32, tag="mx")
                nc.vector.tensor_reduce(out=mx[:], in_=yo[:],
                                        axis=mybir.AxisListType.X, op=ALU.max)
                sh = smp.tile([P, CLS], F32, tag="sh")
                nc.vector.tensor_scalar(out=sh[:], in0=yo[:], scalar1=mx[:, 0:1],
                                        scalar2=None, op0=ALU.subtract)
                ex = smp.tile([P, CLS], F32, tag="ex")
                nc.scalar.activation(out=ex[:], in_=sh[:], func=AF.Exp)
                sm = smp.tile([P, 1], F32, tag="sm")
                nc.vector.tensor_reduce(out=sm[:], in_=ex[:],
                                        axis=mybir.AxisListType.X, op=ALU.add)
                # ls = ln(sm), sm in [1, CLS]; no Ln in any ACT table set, so
                # Newton on f(y) = e^y - sm:  y <- y - 1 + sm * e^-y
                ls = smp.tile([P, 1], F32, tag="ls")
                nc.vector.tensor_scalar(out=ls[:], in0=sm[:], scalar1=0.2559,
                                        scalar2=-0.2559, op0=ALU.mult, op1=ALU.add)
                for _ in range(4):
                    en = smp.tile([P, 1], F32, tag="en")
                    nc.scalar.activation(out=en[:], in_=ls[:], func=AF.Exp,
                                         scale=-1.0)
                    pr = smp.tile([P, 1], F32, tag="pr")
                    nc.vector.tensor_tensor(out=pr[:], in0=en[:], in1=sm[:],
                                            op=ALU.mult)
                    ls2 = smp.tile([P, 1], F32, tag="ls")
                    nc.vector.tensor_tensor(out=ls2[:], in0=ls[:], in1=pr[:],
                                            op=ALU.add)
                    ls = ls2
                    nc.vector.tensor_scalar(out=ls[:], in0=ls[:], scalar1=1.0,
                                            scalar2=None, op0=ALU.subtract)
                res = smp.tile([P, CLS], F32, tag="res")
                nc.vector.tensor_scalar(out=res[:], in0=sh[:], scalar1=ls[:, 0:1],
                                        scalar2=None, op0=ALU.subtract)
                nc.sync.dma_start(out.ap()[t * P:(t + 1) * P, :], res[:])
                off += P * W

    nc.compile()
    return nc


def _prep(x, edge_index, W1, b1, W2, b2):
    """Host-side graph preprocessing; returns (Wt schedule, per-core in_maps, order)."""
    x = np.asarray(x, dtype=np.float32)
    ei = np.asarray(edge_index, dtype=np.int64)
    W1 = np.asarray(W1, dtype=np.float32)
    b1 = np.asarray(b1, dtype=np.float32)
    W2 = np.asarray(W2, dtype=np.float32)
    b2 = np.asarray(b2, dtype=np.float32)

    nodes = np.arange(N, dtype=np.int64)
    src_f = np.concatenate([nodes, ei[0]])
    dst_f = np.concatenate([nodes, ei[1]])
    deg = np.bincount(dst_f, minlength=N)  # >= 1 (self-loops)
    dinv = (1.0 / np.sqrt(deg)).astype(np.float32)

    # per-core degree-descending permutation
    order = np.argsort(-deg.reshape(NC, NPC), axis=1, kind="stable")  # [NC, NPC]
    perm_global = (np.arange(NC, dtype=np.int64)[:, None] * NPC + order)  # [NC, NPC]
    pos_of = np.empty(N, np.int64)
    pos_of[perm_global.ravel()] = np.arange(N, dtype=np.int64)

    dpos = pos_of[dst_f]
    spos = pos_of[src_f]
    sidx = np.argsort(dpos, kind="stable")
    dpos_s = dpos[sidx]
    spos_s = spos[sidx]
    cnt = np.bincount(dpos_s, minlength=N)

    cnt_t = cnt.reshape(NC, NPC)
    cnt_pad = np.zeros((NC, NPAD), np.int64)
    cnt_pad[:, :NPC] = cnt_t
    Wt = np.maximum(cnt_pad.reshape(NC, NT, P).max(axis=2).max(axis=0), 1)  # [NT]
    off_t = np.zeros(NT + 1, np.int64)
    np.cumsum(P * Wt, out=off_t[1:])
    S = int(off_t[-1])

    starts = np.zeros(N + 1, np.int64)
    np.cumsum(cnt, out=starts[1:])
    j = np.arange(len(dpos_s), dtype=np.int64) - starts[dpos_s]
    c_arr = dpos_s // NPC
    il = dpos_s % NPC
    t_arr = il // P
    p_arr = il % P
    col_of = off_t // P  # column offset of each tile in the [P, SW] buffer
    col = col_of[t_arr] + j
    SW = S // P
    idx_all = np.full((NC, P, SW), N, dtype=np.int32)
    idx_all[c_arr, p_arr, col] = spos_s.astype(np.int32)

    dinv_perm = dinv[perm_global]  # [NC, NPC]
    dinv_pad = np.ones((NC, NPAD), np.float32)
    dinv_pad[:, :NPC] = dinv_perm
    dinvT = np.ascontiguousarray(
        dinv_pad.reshape(NC, NT, P).transpose(0, 2, 1))  # [NC, P, NT]

    W1_bf = np.ascontiguousarray(W1.astype(NP_BF16))
    W2_bf = np.ascontiguousarray(W2.astype(NP_BF16))
    b1_bc = np.ascontiguousarray(np.broadcast_to(b1[None, :], (P, H)).astype(np.float32))
    b2_bc = np.ascontiguousarray(np.broadcast_to(b2[None, :], (P, CLS)).astype(np.float32))

    in_maps = []
    for c in range(NC):
        xp = np.zeros((NPAD, F), dtype=NP_BF16)
        xp[:NPC] = x[perm_global[c]].astype(NP_BF16)
        in_maps.append({
            "x_in": xp,
            "w1_in": W1_bf,
            "b1_in": b1_bc,
            "w2_in": W2_bf,
            "b2_in": b2_bc,
            "dinv_in": np.ascontiguousarray(dinvT[c]),
            "idx_in": np.ascontiguousarray(idx_all[c]),
        })
    return tuple(int(w) for w in Wt), in_maps, order


def _get_nc(Wt):
    if Wt not in _cache:
        _cache[Wt] = _build(Wt)
    return _cache[Wt]


def run(x, edge_index, W1, b1, W2, b2, trace=False):
    Wt, in_maps, order = _prep(x, edge_index, W1, b1, W2, b2)
    nc = _get_nc(Wt)
    res = bass_utils.run_bass_kernel_spmd(
        nc, in_maps, core_ids=list(range(NC)), trace=trace)
    out_full = np.empty((N, CLS), np.float32)
    for c in range(NC):
        oc = res.results[c]["out"][:NPC]
        out_full[c * NPC + order[c]] = oc
    return out_full, res


def kernel(x, edge_index, W1, b1, W2, b2):
    out_full, _ = run(x, edge_index, W1, b1, W2, b2)
    return out_full
